# revision 18
# baseline (speedup 1.0000x reference)
"""Channel-attention (nn_CAttention) Trainium2 kernel.

Full inputs in, full output out. Data-parallel over batch B=8 across 8
NeuronCores (one batch element per core); the small [C,C] projection weight
is replicated (passed pre-transposed, cast to bf16, as weight prep).

Per-core math (b fixed, head n in [0,8), c=256 channels, s=2048 spatial):
  qh, kh, vh = q[b].reshape(8, 256, 2048) etc (contiguous view)
  qn = qh / ||qh||_row ; kn likewise          (l2 norm along s)
  GT[d, c] = sum_s kn[d,s] qn[c,s]            (= attn^T)
  sig = sigmoid(GT)
  out_h[c, s] = sum_d sig[d, c] vh[d, s]
  X[32n+j, q*2048+s] = out_h[8j+q, s]         (head -> original channel layout)
  O = W @ X                                   (1x1 conv projection)

Implementation notes:
 - q/k/v are cast fp32->bf16 during the SWDGE DMA load.
 - The [s,c] transposes of q,k are regular matmuls against diag(1/norm),
   so normalization is fused into the transpose for free.
 - The sigmoid writes its output with a permuted free-dim AP (c_new=8j+q
   stored at offset q*32+j), and the out-matmuls are column-packed with
   tile_position=(0,32*hn) so each head group's rank-32 contribution lands
   at the correct PSUM partition base; X then materializes in SBUF already
   in the original-channel layout the projection needs.
"""

import os

os.environ.setdefault("JAX_PLATFORMS", "axon,cpu")

import numpy as np
import ml_dtypes
from contextlib import ExitStack

import concourse.bass as bass
import concourse.tile as tile
from concourse import mybir
from concourse._compat import with_exitstack
from concourse.bass import ts, ds
from concourse.bass_utils import run_bass_kernel_spmd
from concourse.masks import make_identity
from concourse.vector_clock import ScopedClock

B, C, HH, WW = 8, 256, 128, 128
NH = 8
S = (HH * WW) // NH  # 2048
HW = HH * WW  # 16384
EPS = 1e-12

F32 = mybir.dt.float32
BF16 = mybir.dt.bfloat16
AF = mybir.ActivationFunctionType

_MAX_DRAIN_WAITS = 1


def _install_drain_patch():
    """This walrus build rejects >1 sync wait on a CTRL instruction; spread
    the TileContext final-drain waits across chained wait-nops on SP."""

    def _drain_and_barrier_split(self, tick_clock, wait_clock):
        nc = self.nc
        drain_inst = nc.sync.drain()
        wait_clock.add_sem_waits(
            drain_inst.ins, ScopedClock({None: tick_clock.global_clock})
        )
        si = drain_inst.ins.sync_info
        waits = list(si.on_wait) if si is not None else []
        if len(waits) > _MAX_DRAIN_WAITS:
            drain_inst.ins.sync_info = mybir.SyncInfo(
                on_wait=waits[:_MAX_DRAIN_WAITS], on_update=[]
            )
            for i in range(_MAX_DRAIN_WAITS, len(waits), _MAX_DRAIN_WAITS):
                nop = nc.sync.nop(nofuse=True, hint="drain_wait_split")
                nop.ins.sync_info = mybir.SyncInfo(
                    on_wait=waits[i : i + _MAX_DRAIN_WAITS], on_update=[]
                )
        nc.all_engine_barrier()
        assert self.sems is not None
        popped = nc._tile_sem_poison_stack.pop()
        assert popped is self._sem_poison
        nc.clear_and_free_semaphores(list(self.sems.allocated().values()))
        nc.all_engine_barrier()

    tile.TileContext._drain_and_barrier = _drain_and_barrier_split


def _split_excess_waits(nc, max_waits=_MAX_DRAIN_WAITS):
    """This walrus build allows only one sync-wait command per instruction;
    hoist extra waits into nofuse NOPs on the same engine just before."""
    n_split = 0
    for f in nc.m.functions:
        for blk in f.blocks:
            il = blk.instructions
            new = []
            for inst in il:
                si = inst.sync_info
                waits = list(si.on_wait) if si is not None else []
                if len(waits) > max_waits:
                    extra, keep = waits[:-max_waits], waits[-max_waits:]
                    for j in range(0, len(extra), max_waits):
                        nop = mybir.InstNoOp(
                            name=f"{inst.name}-wsplit{j}",
                            sync_info=mybir.SyncInfo(
                                on_wait=extra[j : j + max_waits], on_update=[]
                            ),
                            bass_nofuse=True,
                            engine=inst.engine,
                        )
                        new.append(nop)
                    inst.sync_info = mybir.SyncInfo(
                        on_wait=keep, on_update=list(si.on_update)
                    )
                    n_split += 1
                new.append(inst)
            if len(new) != len(il):
                il[:] = new
    return n_split


def _cattn_consts(ctx: ExitStack, tc: tile.TileContext, wt):
    """One-time constants: identity matrix and the transposed projection
    weight (bf16) resident in SBUF."""
    nc = tc.nc
    consts = ctx.enter_context(tc.tile_pool(name="consts", bufs=1))
    ident = consts.tile([128, 128], F32)
    make_identity(nc, ident)
    wt_sb = consts.tile([128, 2, 256], BF16)
    nc.sync.dma_start(out=wt_sb, in_=wt[:].rearrange("(ch p) o -> p ch o", p=128))
    return ident, wt_sb


@with_exitstack
def _cattn_body(ctx: ExitStack, tc: tile.TileContext, q, k, v, ident, wt_sb, out):
    nc = tc.nc

    natp = ctx.enter_context(tc.tile_pool(name="nat", bufs=3))
    sqs = ctx.enter_context(tc.tile_pool(name="sqs", bufs=2))
    stat = ctx.enter_context(tc.tile_pool(name="stat", bufs=8))
    dpool = ctx.enter_context(tc.tile_pool(name="diag", bufs=8))
    qkt = ctx.enter_context(tc.tile_pool(name="qkt", bufs=2))
    sgp = ctx.enter_context(tc.tile_pool(name="sg", bufs=5))
    vp = ctx.enter_context(tc.tile_pool(name="v", bufs=5))
    xp = ctx.enter_context(tc.tile_pool(name="x", bufs=1))
    obuf = ctx.enter_context(tc.tile_pool(name="obuf", bufs=2))
    tpsum = ctx.enter_context(tc.tile_pool(name="tpsum", bufs=2, space="PSUM"))
    gpsum = ctx.enter_context(tc.tile_pool(name="gpsum", bufs=2, space="PSUM"))
    bpsum = ctx.enter_context(tc.tile_pool(name="bpsum", bufs=3, space="PSUM"))

    X = xp.tile([128, 2, HW], BF16)

    for g in range(2):  # head groups of 4
        sig_tiles = []
        v_tiles = []
        for hn in range(4):
            n = g * 4 + hn
            # ---- load + row l2-norms + normalized transposes of q, k ----
            TT = {}
            for name, src in (("q", q), ("k", k)):
                nats = []
                rrs = []
                for ct in range(2):
                    nat = natp.tile([128, S], BF16, tag=f"{name}nat")
                    nc.gpsimd.dma_start(out=nat, in_=src[n, ts(ct, 128), :])
                    nats.append(nat)
                    scr = sqs.tile([128, S], BF16, tag="scr")
                    ssq = stat.tile([128, 1], F32, tag="ssq")
                    nc.scalar.activation(
                        out=scr, in_=nat, func=AF.Square, accum_out=ssq
                    )
                    nrm = stat.tile([128, 1], F32, tag="nrm")
                    nc.scalar.activation(out=nrm, in_=ssq, func=AF.Sqrt)
                    nc.vector.tensor_scalar_max(out=nrm, in0=nrm, scalar1=EPS)
                    rr = stat.tile([128, 1], F32, tag="rr")
                    nc.vector.reciprocal(out=rr, in_=nrm)
                    rrs.append(rr)
                Dm = []
                for ct in range(2):
                    D = dpool.tile([128, 128], BF16, tag="D")
                    nc.vector.tensor_scalar_mul(out=D, in0=ident, scalar1=rrs[ct])
                    Dm.append(D)
                TTt = qkt.tile([128, 16, 256], BF16, tag=f"{name}T")
                for sbp in range(8):
                    ps = tpsum.tile([128, 512], F32, tag="tps")
                    for half in range(2):
                        sb = sbp * 2 + half
                        for ct in range(2):
                            nc.tensor.matmul(
                                ps[:, ds(half * 256 + ct * 128, 128)],
                                lhsT=nats[ct][:, ts(sb, 128)],
                                rhs=Dm[ct],
                                start=True,
                                stop=True,
                            )
                    nc.vector.tensor_copy(out=TTt[:, sbp * 2 : sbp * 2 + 2], in_=ps)
                TT[name] = TTt

            # ---- v load (bf16 cast) ----
            vt = vp.tile([128, 2, S], BF16, tag="v")
            nc.gpsimd.dma_start(
                out=vt, in_=v[n].rearrange("(a p) s -> p a s", p=128)
            )
            v_tiles.append(vt)

            # ---- GT = KT.T @ QT (normalized), sigmoid with permuted write ----
            sg = sgp.tile([128, 2, 256], BF16, tag="sg")
            for dt_ in range(2):
                gps = gpsum.tile([128, 256], F32, tag="gps")
                for sb in range(16):
                    nc.tensor.matmul(
                        gps,
                        lhsT=TT["k"][:, sb, ts(dt_, 128)],
                        rhs=TT["q"][:, sb],
                        start=(sb == 0),
                        stop=(sb == 15),
                    )
                # c_new = 8j + q_  stored at offset q_*32 + j
                sig_out = sg[:, dt_].rearrange("p (q j) -> p j q", q=8)
                sig_in = gps[:].rearrange("p (j q) -> p j q", q=8)
                nc.scalar.activation(out=sig_out, in_=sig_in, func=AF.Sigmoid)
            sig_tiles.append(sg)

        # ---- out matmuls: col-packed 4 heads -> X[:, g, :] ----
        for q_ in range(8):
            for sc in range(4):
                ps = bpsum.tile([128, 512], F32, tag="ops")
                # col-packed: 4 heads on distinct 32-col groups run
                # concurrently (row tiling is broken on this stack).
                for db in range(2):
                    for hn in range(4):
                        nc.tensor.matmul(
                            ps[ts(hn, 32), :],
                            lhsT=sig_tiles[hn][:, db, ts(q_, 32)],
                            rhs=v_tiles[hn][:, db, ts(sc, 512)],
                            start=(db == 0),
                            stop=(db == 1),
                            tile_position=(0, hn * 32),
                        )
                nc.scalar.copy(
                    out=X[:, g, ds(q_ * S + sc * 512, 512)], in_=ps
                )

    # ---- projection O = W @ X, stream out ----
    for ot in range(2):
        for tb in range(8):
            ob = obuf.tile([128, 2048], F32, tag="ob")
            for tc_ in range(4):
                pps = bpsum.tile([128, 512], F32, tag="ops")
                for ch in range(2):
                    nc.tensor.matmul(
                        pps,
                        lhsT=wt_sb[:, ch, ts(ot, 128)],
                        rhs=X[:, ch, ds(tb * 2048 + tc_ * 512, 512)],
                        start=(ch == 0),
                        stop=(ch == 1),
                    )
                nc.vector.tensor_copy(out=ob[:, ts(tc_, 512)], in_=pps)
            nc.sync.dma_start(out=out[ts(ot, 128), ts(tb, 2048)], in_=ob)


_NC_CACHE = {}


def _build_nc(repeats=1):
    if repeats in _NC_CACHE:
        return _NC_CACHE[repeats]
    _install_drain_patch()
    nc = bass.Bass()
    q = nc.declare_dram_parameter("q", [NH, C, S], F32, isOutput=False)
    k = nc.declare_dram_parameter("k", [NH, C, S], F32, isOutput=False)
    v = nc.declare_dram_parameter("v", [NH, C, S], F32, isOutput=False)
    wt = nc.declare_dram_parameter("wt", [C, C], BF16, isOutput=False)
    out = nc.declare_dram_parameter("out", [C, HW], F32, isOutput=True)
    trace_sim = bool(os.environ.get("TRACE_SIM"))
    with tile.TileContext(nc, trace_sim=trace_sim) as tc:
        with ExitStack() as const_ctx:
            ident, wt_sb = _cattn_consts(const_ctx, tc, wt)
            for _ in range(repeats):
                _cattn_body(tc, q, k, v, ident, wt_sb, out)
    _split_excess_waits(nc)
    _NC_CACHE[repeats] = nc
    return nc


LAST_RESULT = None


def kernel(q, k, v, w_proj):
    global LAST_RESULT
    q = np.ascontiguousarray(np.asarray(q, dtype=np.float32))
    k = np.ascontiguousarray(np.asarray(k, dtype=np.float32))
    v = np.ascontiguousarray(np.asarray(v, dtype=np.float32))
    w_proj = np.asarray(w_proj, dtype=np.float32)

    nc = _build_nc(int(os.environ.get("BENCH_REPEATS", "1")))
    wt = np.ascontiguousarray(w_proj.T).astype(ml_dtypes.bfloat16)
    in_maps = [
        {
            "q": q[b].reshape(NH, C, S),
            "k": k[b].reshape(NH, C, S),
            "v": v[b].reshape(NH, C, S),
            "wt": wt,
        }
        for b in range(B)
    ]
    trace = bool(os.environ.get("BASS_TRACE"))
    res = run_bass_kernel_spmd(nc, in_maps, list(range(B)), trace=trace)
    LAST_RESULT = res
    out = np.stack([np.asarray(res.results[b]["out"]) for b in range(B)])
    return out.reshape(B, C, HH, WW).astype(np.float32)


if __name__ == "__main__":
    rng = np.random.default_rng(0)
    qq = rng.standard_normal((B, C, HH, WW), dtype=np.float32)
    kk = rng.standard_normal((B, C, HH, WW), dtype=np.float32)
    vv = rng.standard_normal((B, C, HH, WW), dtype=np.float32)
    wp = rng.standard_normal((C, C), dtype=np.float32) / np.sqrt(C)
    o = kernel(qq, kk, vv, wp)
    print("out shape:", o.shape, "finite:", np.isfinite(o).all())
